# revision 14
# baseline (speedup 1.0000x reference)
"""Trainium2 Bass kernel for nn_LCAMatrixModel (pairwise selu-MLP grid).

Computes out[i,j] = hard_sigmoid(W2 . selu(A[j] + B[i] + b1) + b2) with
  z = x @ W_enc + b_enc, A = z @ W1[:d], B = z @ W1[d:]
for n=1024, d=128, h=256, distributed over 8 NeuronCores by sharding the
output row dimension i (128 rows per core; x and weights replicated).

Per-core algorithm (all math on device):
  selu(v) = lam*relu(v) + lam*(min(alpha*e^v, alpha) - alpha)
  e^v factorizes: alpha*e^v = P[k,j] * Q[k,i],  P = e^{A^T + ln(alpha)},
  Q = e^{B'^T} with B' = B + b1.  Two fp16 "planes" per (i, k-chunk):
    plane1 = relu(A^T + B'^T[:,i])            (ScalarE, bias-fused)
    plane2 = min(P * Q[:,i], alpha)           (VectorE dual-op tensor_scalar)
  Both are contracted with w = lam*W2/6 on TensorE into a PSUM accumulator
  [128 i, 1024 j] using a sliding-window weight tile (w embedded at column
  128 of a zero [128,256] tile; lhsT window [:,128-i:256-i] routes row i).
  Epilogue: out = min(relu(acc + C), 1), C = 0.5 + (b2 - lam*alpha*sum(W2))/6.
"""

import numpy as np
from contextlib import ExitStack

import concourse.bass as bass
import concourse.bacc as bacc
import concourse.mybir as mybir
from concourse import tile
from concourse import bass_utils

N = 1024
RAW = 128
D = 128
H = 256
N_CORES = 8
IB = N // N_CORES  # 128 output rows per core

LAM = 1.0507009873554804934193349852946
ALPHA = 1.6732632423543772848170429916717

F32 = mybir.dt.float32
F16 = mybir.dt.float16

_CACHE = {}


def build_kernel(n_i=IB, repeat=1):
    AF = mybir.ActivationFunctionType
    OP = mybir.AluOpType

    nc = bacc.Bacc(
        "TRN2",
        target_bir_lowering=False,
        debug=False,
        enable_asserts=False,
        num_devices=N_CORES,
    )
    x_d = nc.dram_tensor("x", [N, RAW], F32, kind="ExternalInput").ap()
    xb_d = nc.dram_tensor("xb", [IB, RAW], F32, kind="ExternalInput").ap()
    we_d = nc.dram_tensor("w_enc", [RAW, D], F32, kind="ExternalInput").ap()
    be_d = nc.dram_tensor("b_enc", [D, 1], F32, kind="ExternalInput").ap()
    w1_d = nc.dram_tensor("w1", [2 * D, H], F32, kind="ExternalInput").ap()
    b1_d = nc.dram_tensor("b1", [H, 1], F32, kind="ExternalInput").ap()
    w2_d = nc.dram_tensor("w2", [H, 1], F32, kind="ExternalInput").ap()
    b2_d = nc.dram_tensor("b2", [1, 1], F32, kind="ExternalInput").ap()
    id_d = nc.dram_tensor("ident", [128, 128], F32, kind="ExternalInput").ap()
    y_d = nc.dram_tensor("y", [IB, N], F32, kind="ExternalOutput").ap()

    with tile.TileContext(nc) as tc, ExitStack() as ctx:
        const = ctx.enter_context(tc.tile_pool(name="const", bufs=1))
        planes = ctx.enter_context(tc.tile_pool(name="planes", bufs=4))
        accp = ctx.enter_context(tc.tile_pool(name="acc", bufs=1, space="PSUM"))

        # ---------------- prologue (inside its own psum pool scope) ---------
        with tc.tile_pool(name="ppsum", bufs=2, space="PSUM") as pp, tc.tile_pool(
            name="ppsum1", bufs=1, space="PSUM"
        ) as pp1:
            ident = const.tile([128, 128], F32, tag="ident")
            nc.sync.dma_start(ident[:], id_d[:])
            wenc = const.tile([128, 128], F32, tag="wenc")
            nc.sync.dma_start(wenc[:], we_d[:])
            benc = const.tile([128, 1], F32, tag="benc")
            nc.sync.dma_start(benc[:], be_d[:])
            w1a = const.tile([128, 256], F32, tag="w1a")
            nc.sync.dma_start(w1a[:], w1_d[0:128, :])
            w1b = const.tile([128, 256], F32, tag="w1b")
            nc.sync.dma_start(w1b[:], w1_d[128:256, :])
            b1t = []
            for c in range(2):
                t = const.tile([128, 1], F32, tag=f"b1_{c}")
                nc.sync.dma_start(t[:], b1_d[c * 128 : (c + 1) * 128, :])
                b1t.append(t)
            w2t = const.tile([128, 2], F32, tag="w2t")
            for c in range(2):
                nc.sync.dma_start(w2t[:, c : c + 1], w2_d[c * 128 : (c + 1) * 128, :])
            b2t = const.tile([1, 1], F32, tag="b2t")
            nc.sync.dma_start(b2t[:], b2_d[:])
            xsb = const.tile([128, 1024], F32, tag="xsb")
            for t in range(8):
                nc.sync.dma_start(
                    xsb[:, t * 128 : (t + 1) * 128], x_d[t * 128 : (t + 1) * 128, :]
                )
            xbsb = const.tile([128, 128], F32, tag="xbsb")
            nc.sync.dma_start(xbsb[:], xb_d[:])

            # transposes: x^T [raw, n], xb^T [raw, ib]
            xT = const.tile([128, 1024], F32, tag="xT")
            for t in range(8):
                ps = pp.tile([128, 128], F32, tag="tps")
                nc.tensor.transpose(ps[:], xsb[:, t * 128 : (t + 1) * 128], ident[:])
                nc.vector.tensor_copy(xT[:, t * 128 : (t + 1) * 128], ps[:])
            xbT = const.tile([128, 128], F32, tag="xbT")
            ps = pp.tile([128, 128], F32, tag="tps")
            nc.tensor.transpose(ps[:], xbsb[:], ident[:])
            nc.vector.tensor_copy(xbT[:], ps[:])

            # z^T = W_enc^T x^T + b_enc  [d, n];  zb^T likewise [d, ib]
            zT = const.tile([128, 1024], F32, tag="zT")
            for jh in range(2):
                ps = pp.tile([128, 512], F32, tag="zps")
                nc.tensor.matmul(
                    ps[:], wenc[:], xT[:, jh * 512 : (jh + 1) * 512],
                    start=True, stop=True,
                )
                nc.scalar.activation(
                    zT[:, jh * 512 : (jh + 1) * 512], ps[:], AF.Identity, bias=benc[:]
                )
            zbT = const.tile([128, 128], F32, tag="zbT")
            ps = pp.tile([128, 128], F32, tag="tps")
            nc.tensor.matmul(ps[:], wenc[:], xbT[:], start=True, stop=True)
            nc.scalar.activation(zbT[:], ps[:], AF.Identity, bias=benc[:])

            # A^T chunks (fp16) and P = exp(A^T + ln(alpha)) (fp16)
            lnalpha = const.tile([128, 1], F32, tag="lnalpha")
            nc.vector.memset(lnalpha[:], float(np.log(ALPHA)))
            AT, Pt = [], []
            for c in range(2):
                at = const.tile([128, 1024], F16, tag=f"AT{c}")
                p = const.tile([128, 1024], F16, tag=f"P{c}")
                for jh in range(2):
                    ps = pp.tile([128, 512], F32, tag="zps")
                    nc.tensor.matmul(
                        ps[:], w1a[:, c * 128 : (c + 1) * 128],
                        zT[:, jh * 512 : (jh + 1) * 512],
                        start=True, stop=True,
                    )
                    sl = slice(jh * 512, (jh + 1) * 512)
                    nc.scalar.activation(at[:, sl], ps[:], AF.Copy)
                    nc.scalar.activation(
                        p[:, sl], ps[:], AF.Exp, bias=lnalpha[:]
                    )
                AT.append(at)
                Pt.append(p)

            # B'^T = W1b^T zb^T + b1 (fp32) and Q = exp(B'^T) (fp32), [128, IB]
            Bp, Qt = [], []
            for c in range(2):
                bp = const.tile([128, IB], F32, tag=f"Bp{c}")
                q = const.tile([128, IB], F32, tag=f"Q{c}")
                ps = pp.tile([128, IB], F32, tag="tps")
                nc.tensor.matmul(
                    ps[:], w1b[:, c * 128 : (c + 1) * 128], zbT[:],
                    start=True, stop=True,
                )
                nc.scalar.activation(bp[:], ps[:], AF.Identity, bias=b1t[c][:])
                nc.scalar.activation(q[:], ps[:], AF.Exp, bias=b1t[c][:])
                Bp.append(bp)
                Qt.append(q)

            # weight windows: zero [128,64] fp16 with col 32 = lam/6 * w2_c
            # (sliced [:, 32-q:64-q] to route strip-row q in M=32 col-tiling)
            wwin = []
            for c in range(2):
                t = const.tile([128, 64], F16, tag=f"win{c}")
                nc.vector.memset(t[:], 0.0)
                nc.vector.tensor_scalar(
                    t[:, 32:33], w2t[:, c : c + 1], LAM / 6.0, None, OP.mult
                )
                wwin.append(t)
            zw128 = const.tile([128, 128], F16, tag="zw128")
            nc.vector.memset(zw128[:], 0.0)

            # C vector: C = 0.5 + (b2 - lam*alpha*sum(W2))/6, broadcast [128,1]
            ones_col = const.tile([128, 1], F32, tag="ones_col")
            nc.vector.memset(ones_col[:], 1.0)
            ones_row = const.tile([1, 128], F32, tag="ones_row")
            nc.vector.memset(ones_row[:], 1.0)
            sps = pp1.tile([1, 1], F32, tag="sps")
            nc.tensor.matmul(sps[:], w2t[:, 0:1], ones_col[:], start=True, stop=False)
            nc.tensor.matmul(sps[:], w2t[:, 1:2], ones_col[:], start=False, stop=True)
            ssb = const.tile([1, 1], F32, tag="ssb")
            nc.vector.tensor_scalar(
                ssb[:], sps[:], -LAM * ALPHA / 6.0, None, OP.mult
            )
            s2 = const.tile([1, 1], F32, tag="s2")
            nc.vector.tensor_scalar(s2[:], b2t[:], 1.0 / 6.0, 0.5, OP.mult, OP.add)
            s3 = const.tile([1, 1], F32, tag="s3")
            nc.vector.tensor_add(s3[:], ssb[:], s2[:])
            cps = pp1.tile([128, 1], F32, tag="cps")
            nc.tensor.matmul(cps[:], ones_row[:], s3[:], start=True, stop=True)
            cvec = const.tile([128, 1], F32, tag="cvec")
            nc.vector.tensor_copy(cvec[:], cps[:])

        # ---------------- main loop --------------------------------------
        accA = accp.tile([128, 512], F32, tag="accA")
        accB = accp.tile([128, 512], F32, tag="accB")

        assert n_i == IB, "col-tiled main loop requires the full 128 rows"
        n_q = n_i // 4  # 32 quads; quad q handles rows {q, q+32, q+64, q+96}
        n_strip = 4

        def main_body():
            # process 4 rows i = q + 32t concurrently via 128x32 col-tiling;
            # strip t writes PSUM partitions [32t, 32t+32).  Only the very
            # first matmul per bank uses start=True (clears has_written for
            # the whole bank); later strips' first writes land on cleared
            # bits and overwrite, everything else accumulates.
            n_mm = {0: 0, 1: 0}
            total_mm = n_i * 4  # per bank
            act_ctr = 0
            # zero both banks (M=128, zero weights): sets every element's
            # has_written bit so all strip matmuls can accumulate
            for acc in (accA, accB):
                nc.tensor.matmul(
                    acc[:], zw128[:], AT[0][:, 0:512],
                    start=True, stop=False, skip_group_check=True,
                )
            for q in range(n_q):
                pts = [[None, None] for _ in range(n_strip)]  # [t][c] -> (p1,p2)
                for t in range(n_strip):
                    i = q + n_q * t
                    for c in range(2):
                        p1 = planes.tile([128, 1024], F16, tag=f"p1c{c}t{t}")
                        if c == 0 and act_ctr % 8 != 7:
                            nc.scalar.activation(
                                p1[:], AT[c][:], AF.Relu, bias=Bp[c][:, i : i + 1]
                            )
                        else:
                            nc.vector.tensor_scalar(
                                p1[:], AT[c][:], Bp[c][:, i : i + 1],
                                0.0, OP.add, OP.max,
                            )
                        if c == 0:
                            act_ctr += 1
                        p2 = planes.tile([128, 1024], F16, tag=f"p2c{c}t{t}")
                        nc.vector.tensor_scalar(
                            p2[:], Pt[c][:], Qt[c][:, i : i + 1],
                            float(ALPHA), OP.mult, OP.min,
                        )
                        pts[t][c] = (p1, p2)
                for c in range(2):
                    win = wwin[c][:, 32 - q % 32 : 64 - q % 32]
                    for pi in range(2):
                        for bank, acc, sl in (
                            (0, accA, slice(0, 512)),
                            (1, accB, slice(512, 1024)),
                        ):
                            for t in range(n_strip):
                                nc.tensor.matmul(
                                    acc[32 * t : 32 * t + 32, :],
                                    win,
                                    pts[t][c][pi][:, sl],
                                    start=False,
                                    stop=(n_mm[bank] == total_mm - 1),
                                    skip_group_check=True,
                                    tile_position=(0, 32 * t),
                                )
                                n_mm[bank] += 1

        if repeat == 1:
            main_body()
        else:
            with tc.For_i(0, repeat, 1):
                main_body()

        # ---------------- epilogue ---------------------------------------
        outsb = const.tile([128, 1024], F32, tag="outsb")
        nc.scalar.activation(outsb[:, 0:512], accA[:], AF.Relu, bias=cvec[:])
        nc.scalar.activation(outsb[:, 512:1024], accB[:], AF.Relu, bias=cvec[:])
        outf = const.tile([128, 1024], F32, tag="outf")
        nc.vector.tensor_scalar(outf[:], outsb[:], 1.0, None, OP.min)
        nc.sync.dma_start(y_d[:, :], outf[:])

    nc.compile()
    return nc


def get_nc(n_i=IB, repeat=1):
    key = (n_i, repeat)
    if key not in _CACHE:
        _CACHE[key] = build_kernel(n_i, repeat)
    return _CACHE[key]


def make_in_maps(inputs):
    x = np.ascontiguousarray(np.asarray(inputs["x"], dtype=np.float32))
    base = {
        "x": x,
        "w_enc": np.ascontiguousarray(np.asarray(inputs["W_enc"], np.float32)),
        "b_enc": np.asarray(inputs["b_enc"], np.float32).reshape(D, 1).copy(),
        "w1": np.ascontiguousarray(np.asarray(inputs["W1"], np.float32)),
        "b1": np.asarray(inputs["b1"], np.float32).reshape(H, 1).copy(),
        "w2": np.ascontiguousarray(np.asarray(inputs["W2"], np.float32)),
        "b2": np.asarray(inputs["b2"], np.float32).reshape(1, 1).copy(),
        "ident": np.eye(128, dtype=np.float32),
    }
    in_maps = []
    for g in range(N_CORES):
        m = dict(base)
        m["xb"] = np.ascontiguousarray(x[g * IB : (g + 1) * IB])
        in_maps.append(m)
    return in_maps


def run_on_cores(inputs, trace=False, **kwargs):
    nc = get_nc()
    in_maps = make_in_maps(inputs)
    res = bass_utils.run_bass_kernel_spmd(
        nc, in_maps, core_ids=list(range(N_CORES)), trace=trace, **kwargs
    )
    return res


def kernel(**inputs) -> np.ndarray:
    res = run_on_cores(inputs, trace=False)
    out = np.concatenate([res.results[g]["y"] for g in range(N_CORES)], axis=0)
    return out.astype(np.float32)


# ---------------------------------------------------------------------------
# Benchmark support: persistent sharded jit runner (mirrors
# bass2jax.run_bass_via_pjrt's multi-core branch, but reusable across calls
# and optionally chaining K sequential executions inside one dispatch).
# ---------------------------------------------------------------------------


def make_runner(chain=1, n_i=IB, repeat=1):
    import jax
    from jax.sharding import Mesh, PartitionSpec
    from jax.experimental.shard_map import shard_map
    from concourse import bass2jax
    from concourse.bass2jax import _bass_exec_p, install_neuronx_cc_hook

    install_neuronx_cc_hook()
    nc = get_nc(n_i, repeat)

    partition_name = nc.partition_id_tensor.name if nc.partition_id_tensor else None
    in_names, out_names, out_avals = [], [], []
    for alloc in nc.m.functions[0].allocations:
        if not isinstance(alloc, mybir.MemoryLocationSet):
            continue
        name = alloc.memorylocations[0].name
        if alloc.kind == "ExternalInput":
            if name != partition_name:
                in_names.append(name)
        elif alloc.kind == "ExternalOutput":
            out_names.append(name)
            out_avals.append(
                jax.core.ShapedArray(
                    tuple(alloc.tensor_shape), mybir.dt.np(alloc.dtype)
                )
            )
    n_params = len(in_names)
    all_names = in_names + out_names
    if partition_name is not None:
        all_names = all_names + [partition_name]

    def _body(*args):
        operands = list(args)
        if partition_name is not None:
            operands.append(bass2jax.partition_id_tensor())
        outs = _bass_exec_p.bind(
            *operands,
            out_avals=tuple(out_avals),
            in_names=tuple(all_names),
            out_names=tuple(out_names),
            lowering_input_output_aliases=(),
            sim_require_finite=True,
            sim_require_nnan=True,
            nc=nc,
        )
        return tuple(outs)

    def _chained(*args):
        ins = list(args[:n_params])
        zeros = list(args[n_params:])
        y = None
        for _ in range(chain):
            # tiny data-dep on previous output defeats CSE and forces
            # sequential execution of the chained kernel launches
            zs = (
                zeros
                if y is None
                else [z + 0.0 * y[0, 0].astype(z.dtype) for z in zeros]
            )
            (y,) = _body(*ins, *zs)
        return (y,)

    devices = jax.devices()[:N_CORES]
    mesh = Mesh(np.asarray(devices), ("core",))
    spec = PartitionSpec("core")
    n_out = len(out_names)
    fn = jax.jit(
        shard_map(
            _chained,
            mesh=mesh,
            in_specs=(spec,) * (n_params + n_out),
            out_specs=(spec,) * n_out,
            check_rep=False,
        ),
        keep_unused=True,
    )

    def prepare(inputs):
        in_maps = make_in_maps(inputs)
        concat = [
            np.concatenate([np.asarray(m[name]) for m in in_maps], axis=0)
            for name in in_names
        ]
        zeros = [
            np.zeros((N_CORES * a.shape[0], *a.shape[1:]), a.dtype)
            for a in out_avals
        ]
        sharding = jax.sharding.NamedSharding(mesh, spec)
        return [jax.device_put(a, sharding) for a in concat + zeros]

    def run(dev_args):
        (y,) = fn(*dev_args)
        return y

    return prepare, run


# revision 22
# speedup vs baseline: 5235.4616x; 5235.4616x over previous
"""Trainium2 Bass kernel for nn_LCAMatrixModel (pairwise selu-MLP grid).

Computes out[i,j] = hard_sigmoid(W2 . selu(A[j] + B[i] + b1) + b2) with
  z = x @ W_enc + b_enc, A = z @ W1[:d], B = z @ W1[d:]
for n=1024, d=128, h=256, distributed over 8 NeuronCores by sharding the
output row dimension i (128 rows per core; x and weights replicated).

Per-core algorithm (all math on device):
  selu(v) = lam*relu(v) + lam*(min(alpha*e^v, alpha) - alpha)
  e^v factorizes: alpha*e^v = P[k,j] * Q[k,i],  P = e^{A^T + ln(alpha)},
  Q = e^{B'^T} with B' = B + b1.  Two fp16 "planes" per (i, k-chunk):
    plane1 = relu(A^T + B'^T[:,i])            (ScalarE, bias-fused)
    plane2 = min(P * Q[:,i], alpha)           (VectorE dual-op tensor_scalar)
  Both are contracted with w = lam*W2/6 on TensorE into a PSUM accumulator
  [128 i, 1024 j].  Rows are processed four at a time (i = q+32t) using
  128x32 PE column tiling: strip t is an independent M=32 matmul at
  tile_position (0,32t) with its own rhs stream, so the four streams run
  concurrently (~57ns per N=512 matmul vs 216ns serial).  The weights are a
  sliding-window tile (w at column 32 of a zero [128,64] fp16 tile; slice
  [:,32-q:64-q] routes PSUM partition 32t+q).  Exactly one start=True
  matmul per bank zeroes it (M=128, zero weights); everything else
  accumulates via per-element has_written bits.
  Epilogue: out = min(relu(acc + C), 1), C = 0.5 + (b2 - lam*alpha*sum(W2))/6.

  Measured (8 cores, axon trn2): steady-state ~160-166us per full pass,
  l2 rel err 1.38e-4 (fp16 planes/weights; fp32 everywhere else).
"""

import numpy as np
from contextlib import ExitStack

import concourse.bass as bass
import concourse.bacc as bacc
import concourse.mybir as mybir
from concourse import tile
from concourse import bass_utils

N = 1024
RAW = 128
D = 128
H = 256
N_CORES = 8
IB = N // N_CORES  # 128 output rows per core

LAM = 1.0507009873554804934193349852946
ALPHA = 1.6732632423543772848170429916717

F32 = mybir.dt.float32
F16 = mybir.dt.float16

_CACHE = {}


def build_kernel(n_i=IB, repeat=1, probe=None):
    AF = mybir.ActivationFunctionType
    OP = mybir.AluOpType

    nc = bacc.Bacc(
        "TRN2",
        target_bir_lowering=False,
        debug=False,
        enable_asserts=False,
        num_devices=N_CORES,
    )
    x_d = nc.dram_tensor("x", [N, RAW], F32, kind="ExternalInput").ap()
    xb_d = nc.dram_tensor("xb", [IB, RAW], F32, kind="ExternalInput").ap()
    we_d = nc.dram_tensor("w_enc", [RAW, D], F32, kind="ExternalInput").ap()
    be_d = nc.dram_tensor("b_enc", [D, 1], F32, kind="ExternalInput").ap()
    w1_d = nc.dram_tensor("w1", [2 * D, H], F32, kind="ExternalInput").ap()
    b1_d = nc.dram_tensor("b1", [H, 1], F32, kind="ExternalInput").ap()
    w2_d = nc.dram_tensor("w2", [H, 1], F32, kind="ExternalInput").ap()
    b2_d = nc.dram_tensor("b2", [1, 1], F32, kind="ExternalInput").ap()
    id_d = nc.dram_tensor("ident", [128, 128], F32, kind="ExternalInput").ap()
    y_d = nc.dram_tensor("y", [IB, N], F32, kind="ExternalOutput").ap()

    with tile.TileContext(nc) as tc, ExitStack() as ctx:
        const = ctx.enter_context(tc.tile_pool(name="const", bufs=1))
        planes = ctx.enter_context(tc.tile_pool(name="planes", bufs=5))
        accp = ctx.enter_context(tc.tile_pool(name="acc", bufs=1, space="PSUM"))

        # ---------------- prologue (inside its own psum pool scope) ---------
        with tc.tile_pool(name="ppsum", bufs=2, space="PSUM") as pp, tc.tile_pool(
            name="ppsum1", bufs=1, space="PSUM"
        ) as pp1:
            ident = const.tile([128, 128], F32, tag="ident")
            nc.sync.dma_start(ident[:], id_d[:])
            wenc = const.tile([128, 128], F32, tag="wenc")
            nc.sync.dma_start(wenc[:], we_d[:])
            benc = const.tile([128, 1], F32, tag="benc")
            nc.sync.dma_start(benc[:], be_d[:])
            w1a = const.tile([128, 256], F32, tag="w1a")
            nc.sync.dma_start(w1a[:], w1_d[0:128, :])
            w1b = const.tile([128, 256], F32, tag="w1b")
            nc.sync.dma_start(w1b[:], w1_d[128:256, :])
            b1t = []
            for c in range(2):
                t = const.tile([128, 1], F32, tag=f"b1_{c}")
                nc.sync.dma_start(t[:], b1_d[c * 128 : (c + 1) * 128, :])
                b1t.append(t)
            w2t = const.tile([128, 2], F32, tag="w2t")
            for c in range(2):
                nc.sync.dma_start(w2t[:, c : c + 1], w2_d[c * 128 : (c + 1) * 128, :])
            b2t = const.tile([1, 1], F32, tag="b2t")
            nc.sync.dma_start(b2t[:], b2_d[:])
            xsb = const.tile([128, 1024], F32, tag="xsb")
            for t in range(8):
                nc.sync.dma_start(
                    xsb[:, t * 128 : (t + 1) * 128], x_d[t * 128 : (t + 1) * 128, :]
                )
            xbsb = const.tile([128, 128], F32, tag="xbsb")
            nc.sync.dma_start(xbsb[:], xb_d[:])

            # transposes: x^T [raw, n], xb^T [raw, ib]
            xT = const.tile([128, 1024], F32, tag="xT")
            for t in range(8):
                ps = pp.tile([128, 128], F32, tag="tps")
                nc.tensor.transpose(ps[:], xsb[:, t * 128 : (t + 1) * 128], ident[:])
                nc.vector.tensor_copy(xT[:, t * 128 : (t + 1) * 128], ps[:])
            xbT = const.tile([128, 128], F32, tag="xbT")
            ps = pp.tile([128, 128], F32, tag="tps")
            nc.tensor.transpose(ps[:], xbsb[:], ident[:])
            nc.vector.tensor_copy(xbT[:], ps[:])

            # z^T = W_enc^T x^T + b_enc  [d, n];  zb^T likewise [d, ib]
            zT = const.tile([128, 1024], F32, tag="zT")
            for jh in range(2):
                ps = pp.tile([128, 512], F32, tag="zps")
                nc.tensor.matmul(
                    ps[:], wenc[:], xT[:, jh * 512 : (jh + 1) * 512],
                    start=True, stop=True,
                )
                nc.scalar.activation(
                    zT[:, jh * 512 : (jh + 1) * 512], ps[:], AF.Identity, bias=benc[:]
                )
            zbT = const.tile([128, 128], F32, tag="zbT")
            ps = pp.tile([128, 128], F32, tag="tps")
            nc.tensor.matmul(ps[:], wenc[:], xbT[:], start=True, stop=True)
            nc.scalar.activation(zbT[:], ps[:], AF.Identity, bias=benc[:])

            # A^T chunks (fp16) and P = exp(A^T + ln(alpha)) (fp16)
            lnalpha = const.tile([128, 1], F32, tag="lnalpha")
            nc.vector.memset(lnalpha[:], float(np.log(ALPHA)))
            AT, Pt = [], []
            for c in range(2):
                at = const.tile([128, 1024], F16, tag=f"AT{c}")
                p = const.tile([128, 1024], F16, tag=f"P{c}")
                for jh in range(2):
                    ps = pp.tile([128, 512], F32, tag="zps")
                    nc.tensor.matmul(
                        ps[:], w1a[:, c * 128 : (c + 1) * 128],
                        zT[:, jh * 512 : (jh + 1) * 512],
                        start=True, stop=True,
                    )
                    sl = slice(jh * 512, (jh + 1) * 512)
                    nc.scalar.activation(at[:, sl], ps[:], AF.Copy)
                    nc.scalar.activation(
                        p[:, sl], ps[:], AF.Exp, bias=lnalpha[:]
                    )
                AT.append(at)
                Pt.append(p)

            # B'^T = W1b^T zb^T + b1 (fp32) and Q = exp(B'^T) (fp32), [128, IB]
            Bp, Qt = [], []
            for c in range(2):
                bp = const.tile([128, IB], F32, tag=f"Bp{c}")
                q = const.tile([128, IB], F32, tag=f"Q{c}")
                ps = pp.tile([128, IB], F32, tag="tps")
                nc.tensor.matmul(
                    ps[:], w1b[:, c * 128 : (c + 1) * 128], zbT[:],
                    start=True, stop=True,
                )
                nc.scalar.activation(bp[:], ps[:], AF.Identity, bias=b1t[c][:])
                nc.scalar.activation(q[:], ps[:], AF.Exp, bias=b1t[c][:])
                Bp.append(bp)
                Qt.append(q)

            # weight windows: zero [128,64] fp16 with col 32 = lam/6 * w2_c
            # (sliced [:, 32-q:64-q] to route strip-row q in M=32 col-tiling)
            wwin = []
            for c in range(2):
                t = const.tile([128, 64], F16, tag=f"win{c}")
                nc.vector.memset(t[:], 0.0)
                nc.vector.tensor_scalar(
                    t[:, 32:33], w2t[:, c : c + 1], LAM / 6.0, None, OP.mult
                )
                wwin.append(t)
            zw128 = const.tile([128, 128], F16, tag="zw128")
            nc.vector.memset(zw128[:], 0.0)

            # C vector: C = 0.5 + (b2 - lam*alpha*sum(W2))/6, broadcast [128,1]
            ones_col = const.tile([128, 1], F32, tag="ones_col")
            nc.vector.memset(ones_col[:], 1.0)
            ones_row = const.tile([1, 128], F32, tag="ones_row")
            nc.vector.memset(ones_row[:], 1.0)
            sps = pp1.tile([1, 1], F32, tag="sps")
            nc.tensor.matmul(sps[:], w2t[:, 0:1], ones_col[:], start=True, stop=False)
            nc.tensor.matmul(sps[:], w2t[:, 1:2], ones_col[:], start=False, stop=True)
            ssb = const.tile([1, 1], F32, tag="ssb")
            nc.vector.tensor_scalar(
                ssb[:], sps[:], -LAM * ALPHA / 6.0, None, OP.mult
            )
            s2 = const.tile([1, 1], F32, tag="s2")
            nc.vector.tensor_scalar(s2[:], b2t[:], 1.0 / 6.0, 0.5, OP.mult, OP.add)
            s3 = const.tile([1, 1], F32, tag="s3")
            nc.vector.tensor_add(s3[:], ssb[:], s2[:])
            cps = pp1.tile([128, 1], F32, tag="cps")
            nc.tensor.matmul(cps[:], ones_row[:], s3[:], start=True, stop=True)
            cvec = const.tile([128, 1], F32, tag="cvec")
            nc.vector.tensor_copy(cvec[:], cps[:])

        # ---------------- main loop --------------------------------------
        accA = accp.tile([128, 512], F32, tag="accA")
        accB = accp.tile([128, 512], F32, tag="accB")

        assert n_i == IB, "col-tiled main loop requires the full 128 rows"
        n_q = n_i // 4  # 32 quads; quad q handles rows {q, q+32, q+64, q+96}
        n_strip = 4

        def main_body():
            # process 4 rows i = q + 32t concurrently via 128x32 col-tiling;
            # strip t writes PSUM partitions [32t, 32t+32).  Only the very
            # first matmul per bank uses start=True (clears has_written for
            # the whole bank); later strips' first writes land on cleared
            # bits and overwrite, everything else accumulates.
            n_mm = {0: 0, 1: 0}
            total_mm = n_i * 4  # per bank
            act_ctr = 0
            # zero both banks (M=128, zero weights): sets every element's
            # has_written bit so all strip matmuls can accumulate
            for acc in (accA, accB):
                nc.tensor.matmul(
                    acc[:], zw128[:], AT[0][:, 0:512],
                    start=True, stop=False, skip_group_check=True,
                )
            for q in range(n_q):
                pts = [[None, None] for _ in range(n_strip)]  # [t][c] -> (p1,p2)
                for t in range(n_strip):
                    i = q + n_q * t
                    for c in range(2):
                        if probe == "noplanes":
                            pts[t][c] = (AT[c], Pt[c])
                            continue
                        p1 = planes.tile([128, 1024], F16, tag=f"p1c{c}t{t}")
                        # ACT takes ~4.5 of the 16 plane tiles per quad
                        # (ACT ~1046ns vs DVE ~411ns per tile -> balance):
                        # all 4 c=0 tiles + every 8th c=1 tile
                        act_take = (c == 0) or (act_ctr % 8 == 0)
                        if act_take:
                            nc.scalar.activation(
                                p1[:], AT[c][:], AF.Relu, bias=Bp[c][:, i : i + 1]
                            )
                        else:
                            nc.vector.tensor_scalar(
                                p1[:], AT[c][:], Bp[c][:, i : i + 1],
                                0.0, OP.add, OP.max,
                            )
                        if c == 1:
                            act_ctr += 1
                        p2 = planes.tile([128, 1024], F16, tag=f"p2c{c}t{t}")
                        nc.vector.tensor_scalar(
                            p2[:], Pt[c][:], Qt[c][:, i : i + 1],
                            float(ALPHA), OP.mult, OP.min,
                        )
                        pts[t][c] = (p1, p2)
                for c in range(2):
                    if probe == "nomm":
                        continue
                    win = wwin[c][:, 32 - q % 32 : 64 - q % 32]
                    for pi in range(2):
                        for bank, acc, sl in (
                            (0, accA, slice(0, 512)),
                            (1, accB, slice(512, 1024)),
                        ):
                            for t in range(n_strip):
                                nc.tensor.matmul(
                                    acc[32 * t : 32 * t + 32, :],
                                    win,
                                    pts[t][c][pi][:, sl],
                                    start=False,
                                    stop=(n_mm[bank] == total_mm - 1),
                                    skip_group_check=True,
                                    tile_position=(0, 32 * t),
                                )
                                n_mm[bank] += 1

        if repeat == 1:
            main_body()
        else:
            with tc.For_i(0, repeat, 1):
                main_body()

        # ---------------- epilogue ---------------------------------------
        outsb = const.tile([128, 1024], F32, tag="outsb")
        nc.scalar.activation(outsb[:, 0:512], accA[:], AF.Relu, bias=cvec[:])
        nc.scalar.activation(outsb[:, 512:1024], accB[:], AF.Relu, bias=cvec[:])
        outf = const.tile([128, 1024], F32, tag="outf")
        nc.vector.tensor_scalar(outf[:], outsb[:], 1.0, None, OP.min)
        nc.sync.dma_start(y_d[:, :], outf[:])

    nc.compile()
    return nc


def get_nc(n_i=IB, repeat=1, probe=None):
    key = (n_i, repeat, probe)
    if key not in _CACHE:
        _CACHE[key] = build_kernel(n_i, repeat, probe)
    return _CACHE[key]


def make_in_maps(inputs):
    x = np.ascontiguousarray(np.asarray(inputs["x"], dtype=np.float32))
    base = {
        "x": x,
        "w_enc": np.ascontiguousarray(np.asarray(inputs["W_enc"], np.float32)),
        "b_enc": np.asarray(inputs["b_enc"], np.float32).reshape(D, 1).copy(),
        "w1": np.ascontiguousarray(np.asarray(inputs["W1"], np.float32)),
        "b1": np.asarray(inputs["b1"], np.float32).reshape(H, 1).copy(),
        "w2": np.ascontiguousarray(np.asarray(inputs["W2"], np.float32)),
        "b2": np.asarray(inputs["b2"], np.float32).reshape(1, 1).copy(),
        "ident": np.eye(128, dtype=np.float32),
    }
    in_maps = []
    for g in range(N_CORES):
        m = dict(base)
        m["xb"] = np.ascontiguousarray(x[g * IB : (g + 1) * IB])
        in_maps.append(m)
    return in_maps


def run_on_cores(inputs, trace=False, **kwargs):
    nc = get_nc()
    in_maps = make_in_maps(inputs)
    res = bass_utils.run_bass_kernel_spmd(
        nc, in_maps, core_ids=list(range(N_CORES)), trace=trace, **kwargs
    )
    return res


def kernel(**inputs) -> np.ndarray:
    # The axon tunnel occasionally drops the first execution right after a
    # long client-side neuronxcc compile ("mesh desynced ... unrecoverable");
    # a short pause + retry recovers once the terminal worker restarts.
    last_err = None
    for attempt in range(3):
        try:
            res = run_on_cores(inputs, trace=False)
            out = np.concatenate(
                [res.results[g]["y"] for g in range(N_CORES)], axis=0
            )
            return out.astype(np.float32)
        except Exception as e:  # noqa: BLE001
            last_err = e
            import time as _time

            _time.sleep(5.0 * (attempt + 1))
    raise last_err


# ---------------------------------------------------------------------------
# Benchmark support: persistent sharded jit runner (mirrors
# bass2jax.run_bass_via_pjrt's multi-core branch, but reusable across calls
# and optionally chaining K sequential executions inside one dispatch).
# ---------------------------------------------------------------------------


def make_runner(chain=1, n_i=IB, repeat=1, probe=None):
    nc = get_nc(n_i, repeat, probe)
    return make_runner_for(nc)


def make_runner_for(nc, n_cores=N_CORES):
    import jax
    from jax.sharding import Mesh, PartitionSpec
    from jax.experimental.shard_map import shard_map
    from concourse import bass2jax
    from concourse.bass2jax import _bass_exec_p, install_neuronx_cc_hook

    install_neuronx_cc_hook()

    partition_name = nc.partition_id_tensor.name if nc.partition_id_tensor else None
    in_names, out_names, out_avals = [], [], []
    for alloc in nc.m.functions[0].allocations:
        if not isinstance(alloc, mybir.MemoryLocationSet):
            continue
        name = alloc.memorylocations[0].name
        if alloc.kind == "ExternalInput":
            if name != partition_name:
                in_names.append(name)
        elif alloc.kind == "ExternalOutput":
            out_names.append(name)
            out_avals.append(
                jax.core.ShapedArray(
                    tuple(alloc.tensor_shape), mybir.dt.np(alloc.dtype)
                )
            )
    n_params = len(in_names)
    all_names = in_names + out_names
    if partition_name is not None:
        all_names = all_names + [partition_name]

    def _body(*args):
        operands = list(args)
        if partition_name is not None:
            operands.append(bass2jax.partition_id_tensor())
        outs = _bass_exec_p.bind(
            *operands,
            out_avals=tuple(out_avals),
            in_names=tuple(all_names),
            out_names=tuple(out_names),
            lowering_input_output_aliases=(),
            sim_require_finite=True,
            sim_require_nnan=True,
            nc=nc,
        )
        return tuple(outs)

    devices = jax.devices()[:n_cores]
    mesh = Mesh(np.asarray(devices), ("core",))
    spec = PartitionSpec("core")
    n_out = len(out_names)
    fn = jax.jit(
        shard_map(
            _body,
            mesh=mesh,
            in_specs=(spec,) * (n_params + n_out),
            out_specs=(spec,) * n_out,
            check_rep=False,
        ),
        keep_unused=True,
    )

    def prepare_maps(in_maps):
        concat = [
            np.concatenate([np.asarray(m[name]) for m in in_maps], axis=0)
            for name in in_names
        ]
        zeros = [
            np.zeros((n_cores * a.shape[0], *a.shape[1:]), a.dtype)
            for a in out_avals
        ]
        sharding = jax.sharding.NamedSharding(mesh, spec)
        return [jax.device_put(a, sharding) for a in concat + zeros]

    def prepare(inputs):
        return prepare_maps(make_in_maps(inputs))

    def run(dev_args):
        outs = fn(*dev_args)
        return outs[0]

    run.prepare_maps = prepare_maps
    return prepare, run
